# revision 1
# baseline (speedup 1.0000x reference)
"""Trainium2 Bass kernel for a CustomGRUCell.

reference:
    r = sigmoid(x @ W_ir.T + b_ir + h @ W_hr.T)
    z = sigmoid(x @ W_iz.T + b_iz + h @ W_hz.T)
    n = tanh(x @ W_in.T + b_in + (r * h) @ W_hn.T)
    h_t = (1 - z) * n + z * h
    returns (h_t, r, z, n)

Shapes: x,h [8192, 2048]; W_* [2048, 2048]; b_* [2048]. All float32.

Strategy: data-parallel over the batch dim (1024 rows per core, 8 cores),
weights replicated. All compute happens in the "transposed world":
the host packs x^T, h^T and W^T so the contraction dim (IN / H-col) lands
on SBUF partitions for both matmul operands; outputs come back as
gate^T [H, B_shard] and are untransposed on the host. This removes every
on-device transpose.

Per-core device schedule (M-tile = 128 rows of H, N chunk = 512 batch cols,
K subtile = 128):
  phase 1 (r): psum = sum_k W_ir^T[k,m] x^T[k,n] + sum_k W_hr^T h^T
               r = sigmoid(psum + b_ir)   -> DRAM
               rh = r * h^T               -> DRAM scratch
  phase 2 (n): psum = x-gemm + rh-gemm; n = tanh(psum + b_in) -> DRAM
  phase 3 (z): psum = x-gemm + h-gemm;  z = sigmoid(psum + b_iz) -> DRAM
               h_t = n + z*(h - n)        -> DRAM (n reloaded from DRAM)

Matmuls run with float32r operands (full PE rate for moving dim >= 256,
fp32 bits, reduced-precision multiply) accumulating in fp32 PSUM.
"""

import numpy as np

import concourse.bass as bass
import concourse.bacc as bacc
import concourse.mybir as mybir
import concourse.tile as tile
from concourse.bass_utils import run_bass_kernel_spmd

F32 = mybir.dt.float32
F32R = mybir.dt.float32r
AFT = mybir.ActivationFunctionType

# Problem constants (hardcoded per contract).
B_FULL = 8192
IN = 2048
H = 2048
N_CORES = 8
BS = B_FULL // N_CORES  # 1024 batch rows per core
P = 128
KO_IN = IN // P  # 16 contraction subtiles for x-gemms
KO_H = H // P    # 16 contraction subtiles for h/rh-gemms
MT = H // P      # 16 output row tiles
NFREE = 512      # moving free dim per matmul (1 PSUM bank of fp32)
NB = BS // NFREE  # 2 batch chunks per core

# Set by the test harness to capture an NTFF profile.
TRACE = False
LAST_RESULTS = None


def _r(ap):
    return ap.bitcast(F32R)


def _build_nc():
    nc = bacc.Bacc("TRN2", target_bir_lowering=False, debug=False)

    xT = nc.dram_tensor("xT", [P, KO_IN, BS], F32, kind="ExternalInput").ap()
    hT = nc.dram_tensor("hT", [P, KO_H, BS], F32, kind="ExternalInput").ap()
    w = {
        name: nc.dram_tensor(name, [MT, P, KO_IN * P], F32, kind="ExternalInput").ap()
        for name in ("w_ir", "w_hr", "w_iz", "w_hz", "w_in", "w_hn")
    }
    b = {
        name: nc.dram_tensor(name, [P, MT], F32, kind="ExternalInput").ap()
        for name in ("b_ir", "b_iz", "b_in")
    }
    outs = {
        name: nc.dram_tensor(name, [MT, P, BS], F32, kind="ExternalOutput").ap()
        for name in ("rT", "zT", "nT", "htT")
    }

    with tile.TileContext(nc) as tc:
        with (
            tc.tile_pool(name="xres", bufs=1) as x_pool,
            tc.tile_pool(name="acts", bufs=1) as a_pool,
            tc.tile_pool(name="wstream", bufs=7) as w_pool,
            tc.tile_pool(name="gates", bufs=8) as g_pool,
            tc.tile_pool(name="bias", bufs=1) as b_pool,
            tc.tile_pool(name="psum", bufs=8, space="PSUM") as ps_pool,
            tc.tile_pool(name="dram", bufs=1, space="DRAM") as d_pool,
        ):
            bias_sb = {}
            for name in ("b_ir", "b_iz", "b_in"):
                t = b_pool.tile([P, MT], F32, tag=name)
                nc.sync.dma_start(t[:], b[name][:])
                bias_sb[name] = t

            rh_dram = d_pool.tile([P, KO_H, BS], F32, tag="rh")

            BLK = 3  # m-tiles in the streaming head block of each phase

            def w_tile(w_ap, mt, nm):
                t = w_pool.tile([P, KO_IN * P], F32, tag="w", name=nm)
                nc.sync.dma_start(_r(t[:]), _r(w_ap[mt]))
                return t

            # x^T stays resident in SBUF for all three phases. The loads are
            # emitted as f32r-dtyped copies: walrus requires the producer of
            # an FP32r matmul operand to carry the float32r output dtype.
            # Head-block weights for phase 1 are interleaved into the x/h
            # per-ko DMA stream so the PE can start consuming immediately.
            x_sb = x_pool.tile([P, KO_IN, BS], F32, tag="x")
            h_sb = a_pool.tile([P, KO_H, BS], F32, tag="act", name="h_sb")
            pre1a, pre1b = {}, {}
            for ko in range(KO_IN):
                if ko < BLK:
                    pre1a[ko] = w_tile(w["w_ir"], ko, f"wa{ko}")
                    pre1b[ko] = w_tile(w["w_hr"], ko, f"wb{ko}")
                nc.sync.dma_start(_r(x_sb[:, ko, :]), _r(xT[:, ko, :]))
                nc.sync.dma_start(_r(h_sb[:, ko, :]), _r(hT[:, ko, :]))

            def load_act(src_ap, name):
                t = a_pool.tile([P, KO_H, BS], F32, tag="act", name=name)
                for ko in range(KO_H):
                    nc.sync.dma_start(_r(t[:, ko, :]), _r(src_ap[:, ko, :]))
                return t

            def phase(wa_ap, wb_ap, rhs_a, rhs_b, consume, interleave_ab,
                      preload=None, pre_mt=None):
                """Head block (first BLK m-tiles): ko-loop OUTER so the PE
                consumes streaming rhs tiles in DMA arrival order (kills
                startup / phase-transition stalls). interleave_ab pairs A/B
                at each ko (phase 1: x and h arrive interleaved); otherwise
                all A first (B source still streaming at phase start).
                Remaining m-tiles: mt-wise with sequential per-bank
                accumulation for smooth PSUM handoff. pre_mt(mt, nb) emits
                prefetches (e.g. n reload) and returns extra consume args."""
                mts = list(range(0, BLK))
                units = [(mt, nb) for mt in mts for nb in range(NB)]
                wa, wb = preload if preload else ({}, {})
                ps = {
                    u: ps_pool.tile(
                        [P, NFREE], F32, tag="ps", name=f"ps_{u[0]}_{u[1]}")
                    for u in units
                }
                pre = {u: None for u in units}  # head block: no prefetch
                # (prefetching for all 6 units at once would starve g_pool)

                def mm(u, w_t, rhs, ko, start, stop):
                    mt, nb = u
                    nc.tensor.matmul(
                        ps[u][:],
                        _r(w_t[mt][:, ko * P:(ko + 1) * P]),
                        _r(rhs[:, ko, nb * NFREE:(nb + 1) * NFREE]),
                        start=start,
                        stop=stop,
                    )

                if interleave_ab:
                    # mt-major with A/B adjacent: matches the DMA arrival
                    # order wa0,wb0,x_ko,h_ko,wa1,wb1,... at startup
                    for ko in range(KO_IN):
                        for mt in mts:
                            for nb in range(NB):
                                mm((mt, nb), wa, rhs_a, ko, ko == 0, False)
                            for nb in range(NB):
                                mm((mt, nb), wb, rhs_b, ko, False,
                                   ko == KO_H - 1)
                else:
                    for ko in range(KO_IN):
                        for u in units:
                            mm(u, wa, rhs_a, ko, ko == 0, False)
                    for ko in range(KO_H):
                        for u in units:
                            mm(u, wb, rhs_b, ko, False, ko == KO_H - 1)
                for u in units:
                    consume(*u, ps[u], pre[u])

                # steady tail: mt-wise, one PSUM bank at a time
                for mt in range(BLK, MT):
                    wa_t = w_tile(wa_ap, mt, f"wa{mt}")
                    wb_t = w_tile(wb_ap, mt, f"wb{mt}")
                    for nb in range(NB):
                        u = (mt, nb)
                        pre_u = pre_mt(mt, nb) if pre_mt else None
                        ps_t = ps_pool.tile(
                            [P, NFREE], F32, tag="ps", name=f"ps_{mt}_{nb}")
                        nbs = slice(nb * NFREE, (nb + 1) * NFREE)
                        for ko in range(KO_IN):
                            nc.tensor.matmul(
                                ps_t[:],
                                _r(wa_t[:, ko * P:(ko + 1) * P]),
                                _r(rhs_a[:, ko, nbs]),
                                start=(ko == 0), stop=False,
                            )
                        for ko in range(KO_H):
                            nc.tensor.matmul(
                                ps_t[:],
                                _r(wb_t[:, ko * P:(ko + 1) * P]),
                                _r(rhs_b[:, ko, nbs]),
                                start=False, stop=(ko == KO_H - 1),
                            )
                        consume(mt, nb, ps_t, pre_u)

            # ---- phase 1: r = sigmoid(x@W_ir^T + b_ir + h@W_hr^T); rh = r*h
            def consume_r(mt, nb, ps_t, _pre):
                nbs = slice(nb * NFREE, (nb + 1) * NFREE)
                r_t = g_pool.tile([P, NFREE], F32, tag="g", name="r_t")
                nc.scalar.activation(
                    r_t[:], ps_t[:], AFT.Sigmoid,
                    bias=bias_sb["b_ir"][:, mt:mt + 1],
                )
                nc.sync.dma_start(outs["rT"][mt][:, nbs], r_t[:])
                rh_t = g_pool.tile([P, NFREE], F32, tag="g", name="rh_t")
                nc.vector.tensor_mul(rh_t[:], r_t[:], h_sb[:, mt, nbs])
                nc.sync.dma_start(_r(rh_dram[:, mt, nbs]), _r(rh_t[:]))

            phase(w["w_ir"], w["w_hr"], x_sb, h_sb, consume_r, True,
                  preload=(pre1a, pre1b))

            # ---- phase 2: n = tanh(x@W_in^T + b_in + rh@W_hn^T)
            # Head-block weights queued before the rh reload so the phase's
            # x-side matmuls can start while rh streams in.
            pre2a = {mt: w_tile(w["w_in"], mt, f"wa{mt}") for mt in range(BLK)}
            pre2b = {mt: w_tile(w["w_hn"], mt, f"wb{mt}") for mt in range(BLK)}
            rh_sb = load_act(rh_dram, "rh_sb")

            def consume_n(mt, nb, ps_t, _pre):
                nbs = slice(nb * NFREE, (nb + 1) * NFREE)
                n_t = g_pool.tile([P, NFREE], F32, tag="g", name="n_t")
                nc.scalar.activation(
                    n_t[:], ps_t[:], AFT.Tanh,
                    bias=bias_sb["b_in"][:, mt:mt + 1],
                )
                nc.sync.dma_start(outs["nT"][mt][:, nbs], n_t[:])

            phase(w["w_in"], w["w_hn"], x_sb, rh_sb, consume_n, False,
                  preload=(pre2a, pre2b))

            # ---- phase 3: z = sigmoid(x@W_iz^T + b_iz + h@W_hz^T)
            #      h_t = n + z*(h - n)
            pre3a = {mt: w_tile(w["w_iz"], mt, f"wa{mt}") for mt in range(BLK)}
            pre3b = {mt: w_tile(w["w_hz"], mt, f"wb{mt}") for mt in range(BLK)}
            h_sb2 = load_act(hT, "h_sb2")

            def prefetch_n(mt, nb):
                # n reload off the critical path: issued before the mt's MMs
                nbs = slice(nb * NFREE, (nb + 1) * NFREE)
                n_t = g_pool.tile([P, NFREE], F32, tag="g", name="n2_t")
                nc.sync.dma_start(n_t[:], outs["nT"][mt][:, nbs])
                return n_t

            def consume_z(mt, nb, ps_t, n_t):
                nbs = slice(nb * NFREE, (nb + 1) * NFREE)
                if n_t is None:
                    n_t = prefetch_n(mt, nb)
                z_t = g_pool.tile([P, NFREE], F32, tag="g", name="z_t")
                nc.scalar.activation(
                    z_t[:], ps_t[:], AFT.Sigmoid,
                    bias=bias_sb["b_iz"][:, mt:mt + 1],
                )
                nc.sync.dma_start(outs["zT"][mt][:, nbs], z_t[:])
                d_t = g_pool.tile([P, NFREE], F32, tag="g", name="d_t")
                nc.vector.tensor_sub(d_t[:], h_sb2[:, mt, nbs], n_t[:])
                nc.vector.tensor_mul(d_t[:], z_t[:], d_t[:])
                ht_t = g_pool.tile([P, NFREE], F32, tag="g", name="ht_t")
                nc.vector.tensor_add(ht_t[:], n_t[:], d_t[:])
                nc.sync.dma_start(outs["htT"][mt][:, nbs], ht_t[:])

            phase(w["w_iz"], w["w_hz"], x_sb, h_sb2, consume_z, False,
                  preload=(pre3a, pre3b), pre_mt=prefetch_n)

    nc.finalize()
    return nc


_NC = None


def _get_nc():
    global _NC
    if _NC is None:
        _NC = _build_nc()
    return _NC


def _round_fp32r(a):
    """Round fp32 to the fp32r grid (11 mantissa bits, RNE). The PE reads
    fp32r operands with the low 12 mantissa bits dropped; pre-rounding on the
    host makes that truncation an exact round-to-nearest."""
    u = np.asarray(a, dtype=np.float32).view(np.uint32)
    lsb = (u >> np.uint32(12)) & np.uint32(1)
    u2 = (u + np.uint32(0x7FF) + lsb) & np.uint32(0xFFFFF000)
    return u2.view(np.float32)


def _pack_w(W):
    # W [H, IN] -> [MT, P, KO*P] with W_host[mt, p, ko, m] = W[mt*P+m, ko*P+p]
    W = _round_fp32r(np.ascontiguousarray(np.asarray(W, dtype=np.float32)))
    return np.ascontiguousarray(
        W.reshape(MT, P, KO_IN, P).transpose(0, 3, 2, 1)
    ).reshape(MT, P, KO_IN * P)


def _pack_act(a):
    # a [BS, D] -> [P, KO, BS] with a_host[p, ko, b] = a[b, ko*P+p]
    return np.ascontiguousarray(
        _round_fp32r(a).reshape(BS, -1, P).transpose(2, 1, 0)
    )


def _pack_b(bvec):
    # b [H] -> [P, MT] with b_host[p, mt] = b[mt*P+p]
    return np.ascontiguousarray(
        np.asarray(bvec, dtype=np.float32).reshape(MT, P).T
    )


def _unpack(arr):
    # [MT, P, BS] -> [BS, H]
    return np.ascontiguousarray(arr.transpose(2, 0, 1)).reshape(BS, H)


def kernel(x, h, W_ir, b_ir, W_hr, W_iz, b_iz, W_hz, W_in, b_in, W_hn):
    global LAST_RESULTS
    nc = _get_nc()

    x = np.ascontiguousarray(np.asarray(x, dtype=np.float32))
    h = np.ascontiguousarray(np.asarray(h, dtype=np.float32))

    shared = {
        "w_ir": _pack_w(W_ir), "w_hr": _pack_w(W_hr),
        "w_iz": _pack_w(W_iz), "w_hz": _pack_w(W_hz),
        "w_in": _pack_w(W_in), "w_hn": _pack_w(W_hn),
        "b_ir": _pack_b(b_ir), "b_iz": _pack_b(b_iz), "b_in": _pack_b(b_in),
    }
    in_maps = []
    for c in range(N_CORES):
        sl = slice(c * BS, (c + 1) * BS)
        in_maps.append({
            "xT": _pack_act(x[sl]),
            "hT": _pack_act(h[sl]),
            **shared,
        })

    res = run_bass_kernel_spmd(
        nc, in_maps, core_ids=list(range(N_CORES)), trace=TRACE
    )
    LAST_RESULTS = res

    def full(name):
        return np.concatenate(
            [_unpack(res.results[c][name]) for c in range(N_CORES)], axis=0
        )

    return full("htT"), full("rT"), full("zT"), full("nT")



# revision 2
# speedup vs baseline: 1.0911x; 1.0911x over previous
"""Trainium2 Bass kernel for a CustomGRUCell.

reference:
    r = sigmoid(x @ W_ir.T + b_ir + h @ W_hr.T)
    z = sigmoid(x @ W_iz.T + b_iz + h @ W_hz.T)
    n = tanh(x @ W_in.T + b_in + (r * h) @ W_hn.T)
    h_t = (1 - z) * n + z * h
    returns (h_t, r, z, n)

Shapes: x,h [8192, 2048]; W_* [2048, 2048]; b_* [2048]. All float32.

Strategy: data-parallel over the batch dim (1024 rows per core, 8 cores),
weights replicated. All compute happens in the "transposed world":
the host packs x^T, h^T and W^T so the contraction dim (IN / H-col) lands
on SBUF partitions for both matmul operands; outputs come back as
gate^T [H, B_shard] and are untransposed on the host.

All matmul operands are float16 (same 1.0 cycles/row PE rate as fp32r on
TRN2, half the DMA bytes and SBUF footprint; quantization error ~3e-4 vs
the 2e-2 gate). PSUM accumulates in fp32. Gate outputs are written as
fp16 and upcast to fp32 on the host.

Everything is SBUF-resident: x^T, h^T stay loaded for all phases; r*h and
n are produced into resident SBUF buffers (no DRAM scratch round trips,
no h reload for phase 3).

Per-core device schedule (M-tile = 128 rows of H, N chunk = 512 batch
cols, K subtile = 128):
  phase 1 (r): psum = sum_k W_ir^T[k,m] x^T[k,n] + sum_k W_hr^T h^T
               r = sigmoid(psum + b_ir)  -> DRAM (fp16)
               rh = r * h^T              -> SBUF resident
  phase 2 (n): psum = x-gemm + rh-gemm; n = tanh(psum + b_in)
               -> SBUF resident + DRAM (fp16)
  phase 3 (z): psum = x-gemm + h-gemm;  z = sigmoid(psum + b_iz) -> DRAM
               h_t = n + z*(h - n)      -> DRAM (all from SBUF residents)
"""

import numpy as np

import concourse.bass as bass
import concourse.bacc as bacc
import concourse.mybir as mybir
import concourse.tile as tile
from concourse.bass_utils import run_bass_kernel_spmd

F16 = mybir.dt.float16
F32 = mybir.dt.float32
AFT = mybir.ActivationFunctionType

# Problem constants (hardcoded per contract).
B_FULL = 8192
IN = 2048
H = 2048
N_CORES = 8
BS = B_FULL // N_CORES  # 1024 batch rows per core
P = 128
KO_IN = IN // P  # 16 contraction subtiles for x-gemms
KO_H = H // P    # 16 contraction subtiles for h/rh-gemms
MT = H // P      # 16 output row tiles
NFREE = 512      # moving free dim per matmul (1 PSUM bank of fp32)
NB = BS // NFREE  # 2 batch chunks per core

# Set by the test harness to capture an NTFF profile.
TRACE = False
LAST_RESULTS = None


def _build_nc():
    nc = bacc.Bacc("TRN2", target_bir_lowering=False, debug=False)

    xT = nc.dram_tensor("xT", [P, KO_IN, BS], F16, kind="ExternalInput").ap()
    hT = nc.dram_tensor("hT", [P, KO_H, BS], F16, kind="ExternalInput").ap()
    w = {
        name: nc.dram_tensor(name, [MT, P, KO_IN * P], F16, kind="ExternalInput").ap()
        for name in ("w_ir", "w_hr", "w_iz", "w_hz", "w_in", "w_hn")
    }
    b = {
        name: nc.dram_tensor(name, [P, MT], F32, kind="ExternalInput").ap()
        for name in ("b_ir", "b_iz", "b_in")
    }
    outs = {
        name: nc.dram_tensor(name, [MT, P, BS], F16, kind="ExternalOutput").ap()
        for name in ("rT", "zT", "nT", "htT")
    }

    with tile.TileContext(nc) as tc:
        with (
            tc.tile_pool(name="xres", bufs=1) as x_pool,
            tc.tile_pool(name="hres", bufs=1) as h_pool,
            tc.tile_pool(name="rhres", bufs=1) as rh_pool,
            tc.tile_pool(name="nres", bufs=1) as n_pool,
            tc.tile_pool(name="wstream", bufs=12) as w_pool,
            tc.tile_pool(name="gates", bufs=8) as g_pool,
            tc.tile_pool(name="bias", bufs=1) as b_pool,
            tc.tile_pool(name="psum", bufs=8, space="PSUM") as ps_pool,
        ):
            bias_sb = {}
            for name in ("b_ir", "b_iz", "b_in"):
                t = b_pool.tile([P, MT], F32, tag=name)
                nc.sync.dma_start(t[:], b[name][:])
                bias_sb[name] = t

            BLK = 3  # m-tiles in the streaming head block of each phase

            def w_tile(w_ap, mt, nm):
                t = w_pool.tile([P, KO_IN * P], F16, tag="w", name=nm)
                nc.sync.dma_start(t[:], w_ap[mt])
                return t

            # Residents. x^T/h^T stay in SBUF for all three phases; rh and n
            # are produced into SBUF by phase 1/2 consumes and read later.
            x_sb = x_pool.tile([P, KO_IN, BS], F16, tag="x")
            h_sb = h_pool.tile([P, KO_H, BS], F16, tag="h")
            rh_sb = rh_pool.tile([P, KO_H, BS], F16, tag="rh")
            n_sb = n_pool.tile([P, MT, BS], F16, tag="n")

            # Phase-1 head-block weights, split so the first k-subtiles land
            # before the whole tile: the PE's first matmuls gate only on a
            # [128,128] chunk + the first x chunk, not on 6 full-width tiles.
            pre1a = {
                mt: w_pool.tile([P, KO_IN * P], F16, tag="w", name=f"wa{mt}")
                for mt in range(BLK)
            }
            pre1b = {
                mt: w_pool.tile([P, KO_IN * P], F16, tag="w", name=f"wb{mt}")
                for mt in range(BLK)
            }

            def head_chunk(ko):
                for mt in range(BLK):
                    nc.sync.dma_start(
                        pre1a[mt][:, ko * P:(ko + 1) * P],
                        w["w_ir"][mt][:, ko * P:(ko + 1) * P])
                    nc.sync.dma_start(
                        pre1b[mt][:, ko * P:(ko + 1) * P],
                        w["w_hr"][mt][:, ko * P:(ko + 1) * P])

            def act_chunk(ko):
                nc.sync.dma_start(x_sb[:, ko, :], xT[:, ko, :])
                nc.sync.dma_start(h_sb[:, ko, :], hT[:, ko, :])

            head_chunk(0)
            act_chunk(0)
            head_chunk(1)
            act_chunk(1)
            act_chunk(2)
            for mt in range(BLK):
                nc.sync.dma_start(pre1a[mt][:, 2 * P:], w["w_ir"][mt][:, 2 * P:])
                nc.sync.dma_start(pre1b[mt][:, 2 * P:], w["w_hr"][mt][:, 2 * P:])
                act_chunk(3 + mt)
            for ko in range(3 + BLK, KO_IN):
                act_chunk(ko)

            def phase(wa_ap, wb_ap, rhs_a, rhs_b, consume, interleave_ab,
                      preload):
                """Head block (first BLK m-tiles): ko-loop OUTER so the PE
                consumes streaming rhs tiles in DMA arrival order (kills
                startup / phase-transition stalls). interleave_ab pairs A/B
                at each ko (phase 1: x and h arrive interleaved); otherwise
                all A first (B source still streaming at phase start).
                Remaining m-tiles: mt-wise, both batch chunks interleaved per
                weight chunk (nb-adjacent matmuls reuse the loaded weights
                back-to-back)."""
                mts = list(range(0, BLK))
                units = [(mt, nb) for mt in mts for nb in range(NB)]
                wa, wb = preload
                ps = {
                    u: ps_pool.tile(
                        [P, NFREE], F32, tag="ps", name=f"ps_{u[0]}_{u[1]}")
                    for u in units
                }

                def mm(u, w_t, rhs, ko, start, stop):
                    mt, nb = u
                    nc.tensor.matmul(
                        ps[u][:],
                        w_t[mt][:, ko * P:(ko + 1) * P],
                        rhs[:, ko, nb * NFREE:(nb + 1) * NFREE],
                        start=start,
                        stop=stop,
                    )

                if interleave_ab:
                    # mt-major with A/B adjacent: matches the DMA arrival
                    # order wa_ko,wb_ko,x_ko,h_ko at startup
                    for ko in range(KO_IN):
                        for mt in mts:
                            for nb in range(NB):
                                mm((mt, nb), wa, rhs_a, ko, ko == 0, False)
                            for nb in range(NB):
                                mm((mt, nb), wb, rhs_b, ko, False,
                                   ko == KO_H - 1)
                else:
                    for ko in range(KO_IN):
                        for u in units:
                            mm(u, wa, rhs_a, ko, ko == 0, False)
                    for ko in range(KO_H):
                        for u in units:
                            mm(u, wb, rhs_b, ko, False, ko == KO_H - 1)
                for u in units:
                    consume(*u, ps[u])

                # steady tail: mt-wise, nb pairs adjacent per weight chunk
                for mt in range(BLK, MT):
                    wa_t = w_tile(wa_ap, mt, f"wa{mt}")
                    wb_t = w_tile(wb_ap, mt, f"wb{mt}")
                    ps_t = [
                        ps_pool.tile(
                            [P, NFREE], F32, tag="ps", name=f"ps_{mt}_{nb}")
                        for nb in range(NB)
                    ]
                    for ko in range(KO_IN):
                        for nb in range(NB):
                            nc.tensor.matmul(
                                ps_t[nb][:],
                                wa_t[:, ko * P:(ko + 1) * P],
                                rhs_a[:, ko, nb * NFREE:(nb + 1) * NFREE],
                                start=(ko == 0), stop=False,
                            )
                    for ko in range(KO_H):
                        for nb in range(NB):
                            nc.tensor.matmul(
                                ps_t[nb][:],
                                wb_t[:, ko * P:(ko + 1) * P],
                                rhs_b[:, ko, nb * NFREE:(nb + 1) * NFREE],
                                start=False, stop=(ko == KO_H - 1),
                            )
                    for nb in range(NB):
                        consume(mt, nb, ps_t[nb])

            # ---- phase 1: r = sigmoid(x@W_ir^T + b_ir + h@W_hr^T); rh = r*h
            def consume_r(mt, nb, ps_t):
                nbs = slice(nb * NFREE, (nb + 1) * NFREE)
                r_t = g_pool.tile([P, NFREE], F16, tag="g", name="r_t")
                nc.scalar.activation(
                    r_t[:], ps_t[:], AFT.Sigmoid,
                    bias=bias_sb["b_ir"][:, mt:mt + 1],
                )
                nc.sync.dma_start(outs["rT"][mt][:, nbs], r_t[:])
                nc.vector.tensor_mul(
                    rh_sb[:, mt, nbs], r_t[:], h_sb[:, mt, nbs])

            phase(w["w_ir"], w["w_hr"], x_sb, h_sb, consume_r, True,
                  preload=(pre1a, pre1b))

            # ---- phase 2: n = tanh(x@W_in^T + b_in + rh@W_hn^T)
            pre2a = {mt: w_tile(w["w_in"], mt, f"wa{mt}") for mt in range(BLK)}
            pre2b = {mt: w_tile(w["w_hn"], mt, f"wb{mt}") for mt in range(BLK)}

            def consume_n(mt, nb, ps_t):
                nbs = slice(nb * NFREE, (nb + 1) * NFREE)
                nc.scalar.activation(
                    n_sb[:, mt, nbs], ps_t[:], AFT.Tanh,
                    bias=bias_sb["b_in"][:, mt:mt + 1],
                )
                nc.sync.dma_start(outs["nT"][mt][:, nbs], n_sb[:, mt, nbs])

            phase(w["w_in"], w["w_hn"], x_sb, rh_sb, consume_n, False,
                  preload=(pre2a, pre2b))

            # ---- phase 3: z = sigmoid(x@W_iz^T + b_iz + h@W_hz^T)
            #      h_t = n + z*(h - n)
            pre3a = {mt: w_tile(w["w_iz"], mt, f"wa{mt}") for mt in range(BLK)}
            pre3b = {mt: w_tile(w["w_hz"], mt, f"wb{mt}") for mt in range(BLK)}

            def consume_z(mt, nb, ps_t):
                nbs = slice(nb * NFREE, (nb + 1) * NFREE)
                z_t = g_pool.tile([P, NFREE], F16, tag="g", name="z_t")
                nc.scalar.activation(
                    z_t[:], ps_t[:], AFT.Sigmoid,
                    bias=bias_sb["b_iz"][:, mt:mt + 1],
                )
                nc.sync.dma_start(outs["zT"][mt][:, nbs], z_t[:])
                d_t = g_pool.tile([P, NFREE], F16, tag="g", name="d_t")
                nc.vector.tensor_sub(
                    d_t[:], h_sb[:, mt, nbs], n_sb[:, mt, nbs])
                nc.vector.tensor_mul(d_t[:], z_t[:], d_t[:])
                ht_t = g_pool.tile([P, NFREE], F16, tag="g", name="ht_t")
                nc.vector.tensor_add(ht_t[:], n_sb[:, mt, nbs], d_t[:])
                nc.sync.dma_start(outs["htT"][mt][:, nbs], ht_t[:])

            phase(w["w_iz"], w["w_hz"], x_sb, h_sb, consume_z, False,
                  preload=(pre3a, pre3b))

    nc.finalize()
    return nc


_NC = None


def _get_nc():
    global _NC
    if _NC is None:
        _NC = _build_nc()
    return _NC


def _pack_w(W):
    # W [H, IN] -> [MT, P, KO*P] with W_host[mt, p, ko, m] = W[mt*P+m, ko*P+p]
    W = np.ascontiguousarray(np.asarray(W, dtype=np.float32))
    return np.ascontiguousarray(
        W.reshape(MT, P, KO_IN, P).transpose(0, 3, 2, 1).astype(np.float16)
    ).reshape(MT, P, KO_IN * P)


def _pack_act(a):
    # a [BS, D] -> [P, KO, BS] with a_host[p, ko, b] = a[b, ko*P+p]
    return np.ascontiguousarray(
        np.asarray(a, dtype=np.float32).reshape(BS, -1, P)
        .transpose(2, 1, 0).astype(np.float16)
    )


def _pack_b(bvec):
    # b [H] -> [P, MT] with b_host[p, mt] = b[mt*P+p]
    return np.ascontiguousarray(
        np.asarray(bvec, dtype=np.float32).reshape(MT, P).T
    )


def _unpack(arr):
    # [MT, P, BS] fp16 -> [BS, H] fp32
    return np.ascontiguousarray(
        arr.astype(np.float32).transpose(2, 0, 1)
    ).reshape(BS, H)


def kernel(x, h, W_ir, b_ir, W_hr, W_iz, b_iz, W_hz, W_in, b_in, W_hn):
    global LAST_RESULTS
    nc = _get_nc()

    x = np.ascontiguousarray(np.asarray(x, dtype=np.float32))
    h = np.ascontiguousarray(np.asarray(h, dtype=np.float32))

    shared = {
        "w_ir": _pack_w(W_ir), "w_hr": _pack_w(W_hr),
        "w_iz": _pack_w(W_iz), "w_hz": _pack_w(W_hz),
        "w_in": _pack_w(W_in), "w_hn": _pack_w(W_hn),
        "b_ir": _pack_b(b_ir), "b_iz": _pack_b(b_iz), "b_in": _pack_b(b_in),
    }
    in_maps = []
    for c in range(N_CORES):
        sl = slice(c * BS, (c + 1) * BS)
        in_maps.append({
            "xT": _pack_act(x[sl]),
            "hT": _pack_act(h[sl]),
            **shared,
        })

    res = run_bass_kernel_spmd(
        nc, in_maps, core_ids=list(range(N_CORES)), trace=TRACE
    )
    LAST_RESULTS = res

    def full(name):
        return np.concatenate(
            [_unpack(res.results[c][name]) for c in range(N_CORES)], axis=0
        )

    return full("htT"), full("rT"), full("zT"), full("nT")


# revision 4
# speedup vs baseline: 1.0970x; 1.0054x over previous
"""Trainium2 Bass kernel for a CustomGRUCell.

reference:
    r = sigmoid(x @ W_ir.T + b_ir + h @ W_hr.T)
    z = sigmoid(x @ W_iz.T + b_iz + h @ W_hz.T)
    n = tanh(x @ W_in.T + b_in + (r * h) @ W_hn.T)
    h_t = (1 - z) * n + z * h
    returns (h_t, r, z, n)

Shapes: x,h [8192, 2048]; W_* [2048, 2048]; b_* [2048]. All float32.

Strategy: data-parallel over the batch dim (1024 rows per core, 8 cores),
weights replicated. All compute happens in the "transposed world":
the host packs x^T, h^T and W^T so the contraction dim (IN / H-col) lands
on SBUF partitions for both matmul operands; outputs come back as
gate^T [H, B_shard] and are untransposed on the host.

All matmul operands are float16 (same 1.0 cycles/row PE rate as fp32r on
TRN2, half the DMA bytes and SBUF footprint; quantization error ~3e-4 vs
the 2e-2 gate). PSUM accumulates in fp32. Gate outputs are written as
fp16 and upcast to fp32 on the host.

Everything is SBUF-resident: x^T, h^T stay loaded for all phases; r*h and
n are produced into resident SBUF buffers (no DRAM scratch round trips,
no h reload for phase 3).

Per-core device schedule (M-tile = 128 rows of H, N chunk = 512 batch
cols, K subtile = 128):
  phase 1 (r): psum = sum_k W_ir^T[k,m] x^T[k,n] + sum_k W_hr^T h^T
               r = sigmoid(psum + b_ir)  -> DRAM (fp16)
               rh = r * h^T              -> SBUF resident
  phase 2 (n): psum = x-gemm + rh-gemm; n = tanh(psum + b_in)
               -> SBUF resident + DRAM (fp16)
  phase 3 (z): psum = x-gemm + h-gemm;  z = sigmoid(psum + b_iz) -> DRAM
               h_t = n + z*(h - n)      -> DRAM (all from SBUF residents)
"""

import numpy as np

import concourse.bass as bass
import concourse.bacc as bacc
import concourse.mybir as mybir
import concourse.tile as tile
from concourse.bass_utils import run_bass_kernel_spmd

F16 = mybir.dt.float16
F32 = mybir.dt.float32
AFT = mybir.ActivationFunctionType

# Problem constants (hardcoded per contract).
B_FULL = 8192
IN = 2048
H = 2048
N_CORES = 8
BS = B_FULL // N_CORES  # 1024 batch rows per core
P = 128
KO_IN = IN // P  # 16 contraction subtiles for x-gemms
KO_H = H // P    # 16 contraction subtiles for h/rh-gemms
MT = H // P      # 16 output row tiles
NFREE = 512      # moving free dim per matmul (1 PSUM bank of fp32)
NB = BS // NFREE  # 2 batch chunks per core

# Set by the test harness to capture an NTFF profile.
TRACE = False
LAST_RESULTS = None


def _build_nc():
    nc = bacc.Bacc("TRN2", target_bir_lowering=False, debug=False)

    xT = nc.dram_tensor("xT", [P, KO_IN, BS], F16, kind="ExternalInput").ap()
    hT = nc.dram_tensor("hT", [P, KO_H, BS], F16, kind="ExternalInput").ap()
    w = {
        name: nc.dram_tensor(name, [MT, P, KO_IN * P], F16, kind="ExternalInput").ap()
        for name in ("w_ir", "w_hr", "w_iz", "w_hz", "w_in", "w_hn")
    }
    b = {
        name: nc.dram_tensor(name, [P, MT], F32, kind="ExternalInput").ap()
        for name in ("b_ir", "b_iz", "b_in")
    }
    outs = {
        name: nc.dram_tensor(name, [MT, P, BS], F16, kind="ExternalOutput").ap()
        for name in ("rT", "zT", "nT", "htT")
    }

    with tile.TileContext(nc) as tc:
        with (
            tc.tile_pool(name="xres", bufs=1) as x_pool,
            tc.tile_pool(name="hres", bufs=1) as h_pool,
            tc.tile_pool(name="rhres", bufs=1) as rh_pool,
            tc.tile_pool(name="nres", bufs=1) as n_pool,
            tc.tile_pool(name="wstream", bufs=12) as w_pool,
            tc.tile_pool(name="gates", bufs=8) as g_pool,
            tc.tile_pool(name="bias", bufs=1) as b_pool,
            tc.tile_pool(name="psum", bufs=8, space="PSUM") as ps_pool,
        ):
            BLK = 3  # m-tiles in the streaming head block of each phase

            def w_tile(w_ap, mt, nm):
                t = w_pool.tile([P, KO_IN * P], F16, tag="w", name=nm)
                nc.sync.dma_start(t[:], w_ap[mt])
                return t

            # Residents. x^T/h^T stay in SBUF for all three phases; rh and n
            # are produced into SBUF by phase 1/2 consumes and read later.
            x_sb = x_pool.tile([P, KO_IN, BS], F16, tag="x")
            h_sb = h_pool.tile([P, KO_H, BS], F16, tag="h")
            rh_sb = rh_pool.tile([P, KO_H, BS], F16, tag="rh")
            n_sb = n_pool.tile([P, MT, BS], F16, tag="n")

            # Phase-1 head-block weights, split so the first k-subtiles land
            # before the whole tile: the PE's first matmuls gate only on a
            # [128,128] chunk + the first x chunk, not on 6 full-width tiles.
            pre1a = {
                mt: w_pool.tile([P, KO_IN * P], F16, tag="w", name=f"wa{mt}")
                for mt in range(BLK)
            }
            pre1b = {
                mt: w_pool.tile([P, KO_IN * P], F16, tag="w", name=f"wb{mt}")
                for mt in range(BLK)
            }

            def head_w(side, mt, kslc):
                dst = (pre1a if side == 0 else pre1b)[mt]
                src = w["w_ir" if side == 0 else "w_hr"][mt]
                nc.sync.dma_start(dst[:, kslc], src[:, kslc])

            def act_chunk(ko):
                nc.sync.dma_start(x_sb[:, ko, :], xT[:, ko, :])
                nc.sync.dma_start(h_sb[:, ko, :], hT[:, ko, :])

            # Startup critical path: the first matmul needs only wa0's first
            # [128,128] chunk + the first half of x's ko=0 chunk, and each
            # sync-queue dispatch costs ~0.6us, so emit exactly in first-use
            # order. Biases aren't needed until the first consume (~25us in).
            k0, k1 = slice(0, P), slice(P, 2 * P)
            head_w(0, 0, k0)
            nc.sync.dma_start(x_sb[:, 0, 0:NFREE], xT[:, 0, 0:NFREE])
            head_w(1, 0, k0)
            nc.sync.dma_start(h_sb[:, 0, 0:NFREE], hT[:, 0, 0:NFREE])
            head_w(0, 1, k0)
            head_w(1, 1, k0)
            head_w(0, 2, k0)
            head_w(1, 2, k0)
            nc.sync.dma_start(x_sb[:, 0, NFREE:], xT[:, 0, NFREE:])
            nc.sync.dma_start(h_sb[:, 0, NFREE:], hT[:, 0, NFREE:])
            for mt in range(BLK):
                head_w(0, mt, k1)
                head_w(1, mt, k1)
            act_chunk(1)

            bias_sb = {}
            for name in ("b_ir", "b_iz", "b_in"):
                t = b_pool.tile([P, MT], F32, tag=name)
                nc.sync.dma_start(t[:], b[name][:])
                bias_sb[name] = t

            act_chunk(2)
            mid, rest = slice(2 * P, 8 * P), slice(8 * P, KO_IN * P)
            for mt in range(BLK):
                head_w(0, mt, mid)
                head_w(1, mt, mid)
                act_chunk(3 + mt)
            for mt in range(BLK):
                head_w(0, mt, rest)
                head_w(1, mt, rest)
                act_chunk(6 + mt)
            for ko in range(6 + BLK, KO_IN):
                act_chunk(ko)

            def phase(wa_ap, wb_ap, rhs_a, rhs_b, consume, interleave_ab,
                      preload):
                """Head block (first BLK m-tiles): ko-loop OUTER so the PE
                consumes streaming rhs tiles in DMA arrival order (kills
                startup / phase-transition stalls). interleave_ab pairs A/B
                at each ko (phase 1: x and h arrive interleaved); otherwise
                all A first (B source still streaming at phase start).
                Remaining m-tiles: mt-wise, both batch chunks interleaved per
                weight chunk (nb-adjacent matmuls reuse the loaded weights
                back-to-back)."""
                mts = list(range(0, BLK))
                units = [(mt, nb) for mt in mts for nb in range(NB)]
                wa, wb = preload
                ps = {
                    u: ps_pool.tile(
                        [P, NFREE], F32, tag="ps", name=f"ps_{u[0]}_{u[1]}")
                    for u in units
                }

                def mm(u, w_t, rhs, ko, start, stop):
                    mt, nb = u
                    nc.tensor.matmul(
                        ps[u][:],
                        w_t[mt][:, ko * P:(ko + 1) * P],
                        rhs[:, ko, nb * NFREE:(nb + 1) * NFREE],
                        start=start,
                        stop=stop,
                    )

                if interleave_ab:
                    # mt-major with A/B adjacent: matches the DMA arrival
                    # order wa_ko,wb_ko,x_ko,h_ko at startup
                    for ko in range(KO_IN):
                        for mt in mts:
                            for nb in range(NB):
                                mm((mt, nb), wa, rhs_a, ko, ko == 0, False)
                            for nb in range(NB):
                                mm((mt, nb), wb, rhs_b, ko, False,
                                   ko == KO_H - 1)
                else:
                    for ko in range(KO_IN):
                        for u in units:
                            mm(u, wa, rhs_a, ko, ko == 0, False)
                    for ko in range(KO_H):
                        for u in units:
                            mm(u, wb, rhs_b, ko, False, ko == KO_H - 1)
                for u in units:
                    consume(*u, ps[u])

                # steady tail: mt-wise, nb pairs adjacent per weight chunk
                for mt in range(BLK, MT):
                    wa_t = w_tile(wa_ap, mt, f"wa{mt}")
                    wb_t = w_tile(wb_ap, mt, f"wb{mt}")
                    ps_t = [
                        ps_pool.tile(
                            [P, NFREE], F32, tag="ps", name=f"ps_{mt}_{nb}")
                        for nb in range(NB)
                    ]
                    for ko in range(KO_IN):
                        for nb in range(NB):
                            nc.tensor.matmul(
                                ps_t[nb][:],
                                wa_t[:, ko * P:(ko + 1) * P],
                                rhs_a[:, ko, nb * NFREE:(nb + 1) * NFREE],
                                start=(ko == 0), stop=False,
                            )
                    for ko in range(KO_H):
                        for nb in range(NB):
                            nc.tensor.matmul(
                                ps_t[nb][:],
                                wb_t[:, ko * P:(ko + 1) * P],
                                rhs_b[:, ko, nb * NFREE:(nb + 1) * NFREE],
                                start=False, stop=(ko == KO_H - 1),
                            )
                    for nb in range(NB):
                        consume(mt, nb, ps_t[nb])

            # ---- phase 1: r = sigmoid(x@W_ir^T + b_ir + h@W_hr^T); rh = r*h
            def consume_r(mt, nb, ps_t):
                nbs = slice(nb * NFREE, (nb + 1) * NFREE)
                r_t = g_pool.tile([P, NFREE], F16, tag="g", name="r_t")
                nc.scalar.activation(
                    r_t[:], ps_t[:], AFT.Sigmoid,
                    bias=bias_sb["b_ir"][:, mt:mt + 1],
                )
                nc.sync.dma_start(outs["rT"][mt][:, nbs], r_t[:])
                nc.vector.tensor_mul(
                    rh_sb[:, mt, nbs], r_t[:], h_sb[:, mt, nbs])

            phase(w["w_ir"], w["w_hr"], x_sb, h_sb, consume_r, True,
                  preload=(pre1a, pre1b))

            # ---- phase 2: n = tanh(x@W_in^T + b_in + rh@W_hn^T)
            pre2a = {mt: w_tile(w["w_in"], mt, f"wa{mt}") for mt in range(BLK)}
            pre2b = {mt: w_tile(w["w_hn"], mt, f"wb{mt}") for mt in range(BLK)}

            def consume_n(mt, nb, ps_t):
                nbs = slice(nb * NFREE, (nb + 1) * NFREE)
                nc.scalar.activation(
                    n_sb[:, mt, nbs], ps_t[:], AFT.Tanh,
                    bias=bias_sb["b_in"][:, mt:mt + 1],
                )
                nc.sync.dma_start(outs["nT"][mt][:, nbs], n_sb[:, mt, nbs])

            phase(w["w_in"], w["w_hn"], x_sb, rh_sb, consume_n, False,
                  preload=(pre2a, pre2b))

            # ---- phase 3: z = sigmoid(x@W_iz^T + b_iz + h@W_hz^T)
            #      h_t = n + z*(h - n)
            pre3a = {mt: w_tile(w["w_iz"], mt, f"wa{mt}") for mt in range(BLK)}
            pre3b = {mt: w_tile(w["w_hz"], mt, f"wb{mt}") for mt in range(BLK)}

            def consume_z(mt, nb, ps_t):
                nbs = slice(nb * NFREE, (nb + 1) * NFREE)
                z_t = g_pool.tile([P, NFREE], F16, tag="g", name="z_t")
                nc.scalar.activation(
                    z_t[:], ps_t[:], AFT.Sigmoid,
                    bias=bias_sb["b_iz"][:, mt:mt + 1],
                )
                nc.sync.dma_start(outs["zT"][mt][:, nbs], z_t[:])
                d_t = g_pool.tile([P, NFREE], F16, tag="g", name="d_t")
                nc.vector.tensor_sub(
                    d_t[:], h_sb[:, mt, nbs], n_sb[:, mt, nbs])
                nc.vector.tensor_mul(d_t[:], z_t[:], d_t[:])
                ht_t = g_pool.tile([P, NFREE], F16, tag="g", name="ht_t")
                nc.vector.tensor_add(ht_t[:], n_sb[:, mt, nbs], d_t[:])
                nc.sync.dma_start(outs["htT"][mt][:, nbs], ht_t[:])

            phase(w["w_iz"], w["w_hz"], x_sb, h_sb, consume_z, False,
                  preload=(pre3a, pre3b))

    nc.finalize()
    return nc


_NC = None


def _get_nc():
    global _NC
    if _NC is None:
        _NC = _build_nc()
    return _NC


def _pack_w(W):
    # W [H, IN] -> [MT, P, KO*P] with W_host[mt, p, ko, m] = W[mt*P+m, ko*P+p]
    W = np.ascontiguousarray(np.asarray(W, dtype=np.float32))
    return np.ascontiguousarray(
        W.reshape(MT, P, KO_IN, P).transpose(0, 3, 2, 1).astype(np.float16)
    ).reshape(MT, P, KO_IN * P)


def _pack_act(a):
    # a [BS, D] -> [P, KO, BS] with a_host[p, ko, b] = a[b, ko*P+p]
    return np.ascontiguousarray(
        np.asarray(a, dtype=np.float32).reshape(BS, -1, P)
        .transpose(2, 1, 0).astype(np.float16)
    )


def _pack_b(bvec):
    # b [H] -> [P, MT] with b_host[p, mt] = b[mt*P+p]
    return np.ascontiguousarray(
        np.asarray(bvec, dtype=np.float32).reshape(MT, P).T
    )


def _unpack(arr):
    # [MT, P, BS] fp16 -> [BS, H] fp32
    return np.ascontiguousarray(
        arr.astype(np.float32).transpose(2, 0, 1)
    ).reshape(BS, H)


def kernel(x, h, W_ir, b_ir, W_hr, W_iz, b_iz, W_hz, W_in, b_in, W_hn):
    global LAST_RESULTS
    nc = _get_nc()

    x = np.ascontiguousarray(np.asarray(x, dtype=np.float32))
    h = np.ascontiguousarray(np.asarray(h, dtype=np.float32))

    shared = {
        "w_ir": _pack_w(W_ir), "w_hr": _pack_w(W_hr),
        "w_iz": _pack_w(W_iz), "w_hz": _pack_w(W_hz),
        "w_in": _pack_w(W_in), "w_hn": _pack_w(W_hn),
        "b_ir": _pack_b(b_ir), "b_iz": _pack_b(b_iz), "b_in": _pack_b(b_in),
    }
    in_maps = []
    for c in range(N_CORES):
        sl = slice(c * BS, (c + 1) * BS)
        in_maps.append({
            "xT": _pack_act(x[sl]),
            "hT": _pack_act(h[sl]),
            **shared,
        })

    res = run_bass_kernel_spmd(
        nc, in_maps, core_ids=list(range(N_CORES)), trace=TRACE
    )
    LAST_RESULTS = res

    def full(name):
        return np.concatenate(
            [_unpack(res.results[c][name]) for c in range(N_CORES)], axis=0
        )

    return full("htT"), full("rT"), full("zT"), full("nT")
